# revision 3
# baseline (speedup 1.0000x reference)
# Trainium2 Bass kernel for nn_DifferenceAttention_38835094291123.
#
# Algebraic structure of the reference network (see its own comments):
#   merge1 = cam_out * ex
#   merge2 = 1 - cam_out * ex
#   taff   = (merge1 + merge2) * 0.5  ==  0.5   for every element
# so every upstream stage (TCN, positional encoding, channel attention, CAM
# gating) is dead code: the output is
#   out[b, 0, l] = sigmoid( l3( l2( l1( 0.5 * ones(64) ) ) ) )
# one scalar broadcast over the whole [B, 1, L] output. Verified numerically:
# the reference output has std == 0 and matches this closed form bit-exactly
# in float32.
#
# The kernel computes the live part of the network on each core — the
# 64->32->16->1 affine head applied to the constant 0.5 vector, plus the
# sigmoid — and broadcast-fills that core's batch shard of the output.
# Sharding is pure data parallel over B (B=32 -> 4 rows per core x 8 cores),
# per the sharding hint; weights are replicated (l3 is also pre-replicated
# across the 128 output partitions so the last matmul lands the scalar in
# every partition directly).
#
# Device program (raw bass, one serial dependency chain, ~6us modeled):
#   SP : DMA-in packed weights -> [mm1 -> +b1 -> mm2 -> +b2 -> mm3(+b3)
#        -> sigmoid -> fill] -> DMA-out the 128KB shard
#   The sigmoid activation-table load is pre-warmed by a dummy activation so
#   it overlaps the input DMA instead of sitting on the critical path; the
#   bias adds run on DVE so only one ACT table set is ever loaded.

import numpy as np

B, L, E = 32, 8192, 64
N_CORES = 8
B_SHARD = B // N_CORES          # 4 batch rows per core
OUT_P, OUT_F = 128, (B_SHARD * L) // 128   # [128, 256] = the 32768-elem shard

_CACHED = None  # compiled bass module, built once per process


def _strip_preamble(nc):
    # Drop the const-AP pool memsets + the entry all-engine barrier that
    # Bass emits unconditionally: this kernel uses no const APs, and all
    # cross-engine ordering is established by explicit semaphores. Best
    # effort — if the internals shift, keeping the barriers is merely slower.
    try:
        bb = nc.main_func.blocks[0]
        bb.instructions[:] = [
            ins for ins in bb.instructions
            if "const-" not in str(ins)
            and "barrier_Pool_Activation_PE_DVE_SP" not in str(ins)
        ]
    except Exception:
        pass


def _strip_end_barrier(nc):
    # Drop the exit all-engine barrier (engine drains are kept). Output
    # landing in DRAM is guaranteed by SP's final wait on the DMA semaphore.
    try:
        for bb in nc.main_func.blocks:
            if not bb.name.endswith("_end"):
                continue
            bb.instructions[:] = [
                ins for ins in bb.instructions
                if "barrier_Pool_Activation_PE_DVE_SP" not in str(ins)
                and "EVENT_SEMAPHORE_RANGE_CLEAR" not in str(ins)
            ]
    except Exception:
        pass


def _build_bass_module():
    from contextlib import ExitStack

    import concourse.bacc as bacc
    import concourse.mybir as mybir

    f32 = mybir.dt.float32
    AF = mybir.ActivationFunctionType

    nc = bacc.Bacc("TRN2", target_bir_lowering=False, debug=False,
                   num_devices=N_CORES)
    _strip_preamble(nc)

    # Packed live parameters, one small f32 tensor [64, 178]:
    #   [:, 0:32]          l1w^T [64, 32]
    #   [0:32, 32:48]      l2w^T [32, 16]
    #   [0:17, 48:176]     rows 0:16 = l3w^T replicated over 128 cols,
    #                      row 16   = l3b replicated (bias via h2 aug row)
    #   [0:32, 176]        l1b
    #   [0:16, 177]        l2b
    pw = nc.dram_tensor("pw", [64, 178], f32, kind="ExternalInput")
    out = nc.dram_tensor("out", [OUT_P, OUT_F], f32, kind="ExternalOutput")

    ctx = ExitStack()
    P = ctx.enter_context(nc.sbuf_tensor("P", [64, 178], f32))
    v05 = ctx.enter_context(nc.sbuf_tensor("v05", [64, 1], f32))
    h1 = ctx.enter_context(nc.sbuf_tensor("h1", [32, 1], f32))
    h2 = ctx.enter_context(nc.sbuf_tensor("h2", [17, 1], f32))
    zc = ctx.enter_context(nc.sbuf_tensor("zc", [128, 1], f32))
    scr = ctx.enter_context(nc.sbuf_tensor("scr", [1, 1], f32))
    sB = ctx.enter_context(nc.sbuf_tensor("sB", [128, 1], f32))
    ob = ctx.enter_context(nc.sbuf_tensor("ob", [OUT_P, OUT_F], f32))
    ps1 = ctx.enter_context(nc.psum_tensor("ps1", [32, 1], f32))
    ps2 = ctx.enter_context(nc.psum_tensor("ps2", [16, 1], f32))
    ps3 = ctx.enter_context(nc.psum_tensor("ps3", [128, 1], f32))
    dS = ctx.enter_context(nc.semaphore("dS"))
    pS = ctx.enter_context(nc.semaphore("pS"))
    vS = ctx.enter_context(nc.semaphore("vS"))
    aS = ctx.enter_context(nc.semaphore("aS"))

    with nc.Block(no_gpsimd_drain=True) as block:

        @block.sync
        def _(sp):
            sp.dma_start(P[:, :], pw[:, :]).then_inc(dS, 16)
            sp.wait_ge(vS, 7)
            sp.dma_start(out[:, :], ob[:, :]).then_inc(dS, 16)
            sp.wait_ge(dS, 32)

        @block.vector
        def _(v):
            nc.vector.memset(v05[:, :], 0.5).then_inc(vS, 1)
            nc.vector.memset(h2[0:17, :], 1.0).then_inc(vS, 1)
            nc.vector.memset(zc[:, :], 0.0).then_inc(vS, 1)
            nc.vector.memset(ob[:, :], 0.0).then_inc(vS, 1)
            v.wait_ge(pS, 1)
            nc.vector.tensor_scalar_add(
                h1[:, :], ps1[:, :], P[0:32, 176:177]).then_inc(vS, 1)
            v.wait_ge(pS, 2)
            nc.vector.tensor_scalar_add(
                h2[0:16, :], ps2[:, :], P[0:16, 177:178]).then_inc(vS, 1)
            v.wait_ge(aS, 1)
            nc.vector.tensor_scalar_add(
                ob[:, :], ob[:, :], sB[:, 0:1]).then_inc(vS, 1)

        @block.tensor
        def _(pe):
            pe.wait_ge(dS, 16)
            pe.wait_ge(vS, 1)
            nc.tensor.matmul(ps1[:, :], P[0:64, 0:32], v05[:, :]).then_inc(pS, 1)
            pe.wait_ge(vS, 5)
            nc.tensor.matmul(ps2[:, :], P[0:32, 32:48], h1[:, :]).then_inc(pS, 1)
            pe.wait_ge(vS, 6)
            nc.tensor.matmul(ps3[:, :], P[0:17, 48:176], h2[0:17, :]).then_inc(pS, 1)

        @block.scalar
        def _(act):
            act.wait_ge(vS, 3)
            # dummy: pulls the sigmoid table-set load off the critical path
            nc.scalar.activation(scr[:, :], zc[0:1, 0:1], AF.Sigmoid,
                                 bias=zc[0:1, 0:1])
            act.wait_ge(pS, 3)
            nc.scalar.activation(sB[:, :], ps3[:, :], AF.Sigmoid,
                                 bias=zc[:, 0:1]).then_inc(aS, 1)

    _strip_end_barrier(nc)
    nc.compile()
    ctx.close()
    return nc


def _get_module():
    global _CACHED
    if _CACHED is None:
        _CACHED = _build_bass_module()
    return _CACHED


def _pack_params(params) -> np.ndarray:
    l1w = np.asarray(params["l1"]["w"], np.float32)[:, :, 0]   # [32, 64]
    l1b = np.asarray(params["l1"]["b"], np.float32)
    l2w = np.asarray(params["l2"]["w"], np.float32)[:, :, 0]   # [16, 32]
    l2b = np.asarray(params["l2"]["b"], np.float32)
    l3w = np.asarray(params["l3"]["w"], np.float32)[:, :, 0]   # [1, 16]
    l3b = np.asarray(params["l3"]["b"], np.float32)
    pw = np.zeros((64, 178), np.float32)
    pw[:, 0:32] = l1w.T
    pw[0:32, 32:48] = l2w.T
    pw[0:16, 48:176] = np.repeat(l3w[0, :, None], 128, axis=1)
    pw[16, 48:176] = l3b[0]
    pw[0:32, 176] = l1b
    pw[0:16, 177] = l2b
    return pw


def kernel(x: np.ndarray, params) -> np.ndarray:
    from concourse.bass_utils import run_bass_kernel_spmd

    nc = _get_module()
    pw = _pack_params(params)
    in_maps = [{"pw": pw} for _ in range(N_CORES)]
    res = run_bass_kernel_spmd(nc, in_maps, core_ids=list(range(N_CORES)))
    shards = [r["out"].reshape(B_SHARD, 1, L) for r in res.results]
    return np.concatenate(shards, axis=0)
